# revision 21
# baseline (speedup 1.0000x reference)
"""Causal self-attention (B=4, T=2048, C=1024, H=16) on 8 Trainium2 cores.

Sharding: core c = (batch b = c // 2, head-group g = c % 2).  Each core
computes 8 of the 16 heads for one batch element (tensor-parallel split of
c_attn columns / c_proj rows) and returns a partial [T, C] output; the host
sums the two head-group partials per batch and adds b_proj.

Per-core dataflow (all matmuls fp32r: fp32 with 12-bit-mantissa RNE inputs,
full PE rate at N>=256, fp32 PSUM accumulation).  The three stages are
interleaved over 512-wide T chunks j -- causality means attention for query
chunk j only needs k/v of chunks <= j, so each chunk's projections feed its
attention and output projection immediately, keeping PE busy while ACT grinds
through the exp()s of earlier chunks:

  per chunk j:
    1) qk^T[:, j] = (x @ w_qk + b_qk)^T   (w stationary, x^T moving; DVE
       eviction adds per-partition bias and writes q chunks / persistent k^T
       tiles directly in SBUF)
       v~[j] = [x @ w_v + b_v | 1]        (x stationary; K=1 ones-row matmul
       adds the bias; the appended ones column later accumulates softmax
       denominators for free)
    2) per head: S^T blocks = k @ q^T (K=64 matmuls), P^T = exp(S^T/8) on
       ACT with causal handling on diagonal blocks (zero-memset prefix +
       upper-triangular mask multiply), y~^T += v~^T @ P^T over key tiles in
       PSUM; rows 0..63 = unnormalized y^T, row 64 = denominator; normalize
       via DVE reciprocal -> K=1 ones broadcast matmul -> DVE multiply.
    3) out[512j:512j+512, :] = y^T.T @ w_proj, streamed to DRAM.
"""

import numpy as np

B, T, C = 4, 2048, 1024
H = 16
HD = C // H          # 64
G = 2                # head groups (cores per batch)
NHG = H // G         # 8 heads per core
CL = NHG * HD        # 512 local channels
P = 128
NK = C // P          # 8 contraction tiles over C
NM = (2 * CL) // P   # 8 row tiles of qk^T
NT = T // P          # 16 T tiles
NJ = 4               # tq chunks
TQ = T // NJ         # 512
NCK = CL // P        # 4 y^T row tiles
SCALE = HD ** -0.5

_STATE = {}


def _round_fp32r(a) -> np.ndarray:
    u = np.ascontiguousarray(a, dtype=np.float32).view(np.uint32)
    r = (u + np.uint32(0x7FF) + ((u >> np.uint32(12)) & np.uint32(1))) \
        & np.uint32(0xFFFFF000)
    return r.view(np.float32)


def _build():
    from concourse import bacc
    import concourse.mybir as mybir
    import concourse.tile as tile

    F32 = mybir.dt.float32
    F32R = mybir.dt.float32r
    EXP = mybir.ActivationFunctionType.Exp

    nc = bacc.Bacc()

    xT = nc.declare_dram_parameter("xT", [C, T], F32R, isOutput=False)
    w_qk = nc.declare_dram_parameter("w_qk", [C, 2 * CL], F32R, isOutput=False)
    w_v = nc.declare_dram_parameter("w_v", [C, CL], F32R, isOutput=False)
    b_qk = nc.declare_dram_parameter("b_qk", [P, NM], F32, isOutput=False)
    w_pr = nc.declare_dram_parameter("w_pr", [CL, C], F32R, isOutput=False)
    tri = nc.declare_dram_parameter("tri", [P, P], F32R, isOutput=False)
    out = nc.declare_dram_parameter("out", [T, C], F32, isOutput=True)

    wqk_k = w_qk.rearrange("(k p) n -> p k n", p=P)  # [128, 8, 1024]

    with tile.TileContext(nc) as tc:
        with (
            tc.tile_pool(name="consts", bufs=1) as consts,
            tc.tile_pool(name="wqk", bufs=1) as wqkp,
            tc.tile_pool(name="wv", bufs=1) as wvp,
            tc.tile_pool(name="wpr", bufs=1) as wprp,
            tc.tile_pool(name="xt", bufs=8) as xtp,
            tc.tile_pool(name="kp", bufs=1) as kpp,
            tc.tile_pool(name="qch", bufs=5) as qchp,
            tc.tile_pool(name="vall", bufs=1) as vallp,
            tc.tile_pool(name="ytc", bufs=8) as ytcp,
            tc.tile_pool(name="pt", bufs=3) as ptp,
            tc.tile_pool(name="rc", bufs=2) as rcp,
            tc.tile_pool(name="rb", bufs=1) as rbp,
            tc.tile_pool(name="ostg", bufs=2) as ostgp,
            tc.tile_pool(name="psum", bufs=1, space="PSUM") as psum,
        ):
            # ---------------- constants ----------------
            ones = consts.tile([1, P], F32R)
            nc.vector.memset(ones.bitcast(F32), 1.0)
            tri_sb = consts.tile([P, P], F32R)
            nc.sync.dma_start(out=tri_sb, in_=tri[:, :])
            bqk_sb = consts.tile([P, NM], F32)
            nc.sync.dma_start(out=bqk_sb, in_=b_qk[:, :])

            # first chunk of x^T, then weights (DMA-order: first matmuls
            # only need slab m=0 + these)
            MORDER = [0, 4, 1, 5, 2, 6, 3, 7]  # pair u's q (m=u), k (m=4+u)
            wqk_sb = [None] * NM

            def load_slab(m):
                wt = wqkp.tile([P, NK, P], F32R, name=f"wqk{m}", tag=f"wqk{m}")
                nc.sync.dma_start(out=wt[:, 0:2, :],
                                  in_=wqk_k[:, 0:2, m * P:(m + 1) * P])
                nc.sync.dma_start(out=wt[:, 2:NK, :],
                                  in_=wqk_k[:, 2:NK, m * P:(m + 1) * P])
                wqk_sb[m] = wt

            load_slab(MORDER[0])
            xts = []
            for k in range(NK):
                xtile = xtp.tile([P, TQ], F32R, name=f"xt_0_{k}", tag="xt")
                nc.sync.dma_start(out=xtile, in_=xT[k * P:(k + 1) * P, 0:TQ])
                xts.append(xtile)
            for m in MORDER[1:]:
                load_slab(m)

            wv_sb = []
            for k in range(NK):
                wvt = wvp.tile([P, CL], F32R, name=f"wv{k}", tag=f"wv{k}")
                nc.sync.dma_start(out=wvt, in_=w_v[k * P:(k + 1) * P, :])
                wv_sb.append(wvt)

            wpr_sb = []
            for ck in range(NCK):
                wpt = wprp.tile([P, C], F32R, name=f"wpr{ck}", tag=f"wpr{ck}")
                nc.sync.dma_start(out=wpt, in_=w_pr[ck * P:(ck + 1) * P, :])
                wpr_sb.append(wpt)

            # persistent k^T tiles (pair u holds heads 2u, 2u+1)
            kp = [kpp.tile([P, T], F32R, name=f"kp{u}", tag=f"kp{u}")
                  for u in range(4)]

            # v~: [128, t, head, 65]; col 64 = ones (denominator trick)
            v_all = vallp.tile([P, NT, NHG, HD + 1], F32R)
            nc.gpsimd.memset(v_all[:, :, :, HD:HD + 1].bitcast(F32), 1.0)

            # ---- emission helpers; engines run their streams in order, so
            # program order performs explicit software pipelining: each
            # attention head (ACT-bound) is followed by next-chunk
            # projection groups and prev-chunk output-projection groups
            # (dependency-ready PE work) as fillers.
            xts_by_j = {0: xts}
            qch_by_j = {}
            ytc_by_j = {}

            def emit_xt_dmas(j):
                lst = []
                for k in range(NK):
                    xtile = xtp.tile([P, TQ], F32R, name=f"xt_{j}_{k}",
                                     tag="xt")
                    nc.sync.dma_start(
                        out=xtile,
                        in_=xT[k * P:(k + 1) * P, j * TQ:(j + 1) * TQ])
                    lst.append(xtile)
                xts_by_j[j] = lst

            def part_a(j, m):
                xts = xts_by_j[j]
                pacc = psum.tile([P, TQ], F32, name=f"pA_{j}_{m}",
                                 tag="acc", bufs=3)
                for k in range(NK):
                    nc.tensor.matmul(pacc, lhsT=wqk_sb[m][:, k, :],
                                     rhs=xts[k], start=(k == 0),
                                     stop=(k == NK - 1))
                if m < 4:  # q rows for pair u=m: chunk-local tile
                    qt = qchp.tile([P, TQ], F32R, name=f"qch_{j}_{m}",
                                   tag="qch")
                    nc.vector.tensor_scalar_add(qt, pacc, bqk_sb[:, m:m + 1])
                    qch_by_j.setdefault(j, [None] * 4)[m] = qt
                else:      # k rows for pair u=m-4: persistent k^T tile
                    nc.vector.tensor_scalar_add(
                        kp[m - 4][:, j * TQ:(j + 1) * TQ], pacc,
                        bqk_sb[:, m:m + 1])

            def part_b(j, ts):
                xts = xts_by_j[j]
                t = j * 4 + ts
                pv = psum.tile([P, CL], F32, name=f"pB_{t}", tag="acc",
                               bufs=3)
                for k in range(NK):
                    nc.tensor.matmul(pv, lhsT=xts[k][:, ts * P:(ts + 1) * P],
                                     rhs=wv_sb[k], start=(k == 0),
                                     stop=(k == NK - 1))
                nc.vector.tensor_copy(
                    v_all[:, t, :, 0:HD],
                    pv.rearrange("p (h d) -> p h d", d=HD))

            def attn_head_main(j, h):
                """ST blocks -> exp -> y~^T accumulation; returns py.

                Key-tile blocks are processed in pairs sharing one 2-bank
                PSUM tile so fully-causal pairs need a single exp op.
                """
                u, s = h // 2, h % 2
                qh = qch_by_j[j][u][HD * s:HD * (s + 1), :]
                kh = kp[u][HD * s:HD * (s + 1), :]
                nr = 4 * (j + 1)
                py = psum.tile([P, TQ], F32, name=f"py_{j}_{h}", tag="py",
                               bufs=1)

                def nlo(r):
                    # first tq column worth computing for key tile r; capped
                    # at 256 so matmuls keep N>=256 (fp32r full-rate)
                    return min(max(r - 4 * j, 0) * P, 256)

                def make_pt_pair(pr):
                    r0 = 2 * pr
                    ps2 = psum.tile([P, 2 * TQ], F32, name=f"ps_{j}_{h}_{pr}",
                                    tag="ps", bufs=2)
                    for half, r in enumerate((r0, r0 + 1)):
                        o = half * TQ
                        n0 = nlo(r)
                        nc.tensor.matmul(ps2[:, o + n0:o + TQ],
                                         lhsT=kh[:, r * P:(r + 1) * P],
                                         rhs=qh[:, n0:TQ], start=True,
                                         stop=True)
                    pt = ptp.tile([P, 2 * TQ], F32R, name=f"pt_{j}_{h}_{pr}",
                                  tag="pt", bufs=2)
                    if r0 + 1 < 4 * j:  # both blocks fully causal: one exp
                        nc.scalar.activation(pt, ps2, EXP, scale=SCALE)
                    else:  # diagonal-straddling pair
                        for half, r in enumerate((r0, r0 + 1)):
                            o = half * TQ
                            a = (r - 4 * j) * P
                            nc.scalar.activation(pt[:, o + a:o + TQ],
                                                 ps2[:, o + a:o + TQ], EXP,
                                                 scale=SCALE)
                            if a > nlo(r):
                                nc.gpsimd.memset(
                                    pt[:, o + nlo(r):o + a].bitcast(F32), 0.0)
                            nc.vector.tensor_mul(pt[:, o + a:o + a + P],
                                                 pt[:, o + a:o + a + P],
                                                 tri_sb)
                    return pt

                pts = make_pt_pair(0)
                for pr in range(nr // 2):
                    pt_next = make_pt_pair(pr + 1) if pr + 1 < nr // 2 else None
                    for half, r in enumerate((2 * pr, 2 * pr + 1)):
                        o = half * TQ
                        n0 = nlo(r)
                        nc.tensor.matmul(py[0:HD + 1, n0:TQ],
                                         lhsT=v_all[:, r, h, :],
                                         rhs=pts[:, o + n0:o + TQ],
                                         start=(r == 0), stop=(r == nr - 1))
                    pts = pt_next
                return py

            def attn_head_finish(j, h, py):
                """normalize: recip -> ones-broadcast matmul -> multiply."""
                u, s = h // 2, h % 2
                rc = rcp.tile([1, TQ], F32R, name=f"rc_{j}_{h}", tag="rc")
                with nc.allow_low_precision("fp32r attention"):
                    nc.vector.reciprocal(rc, py[HD:HD + 1, :])
                pb = psum.tile([P, TQ], F32, name=f"pb_{j}_{h}", tag="acc",
                               bufs=3)
                nc.tensor.matmul(pb[0:HD, :], lhsT=ones[:, 0:HD], rhs=rc,
                                 start=True, stop=True)
                rb = rbp.tile([HD, TQ], F32, name=f"rb_{j}_{h}", tag="rb")
                nc.vector.tensor_copy(rb, pb[0:HD, :])
                nc.vector.tensor_mul(
                    ytc_by_j[j][u][HD * s:HD * (s + 1), :], py[0:HD, :], rb)

            def ph3_group(j, g):
                ts, n2 = g // 2, g % 2
                t = j * 4 + ts
                ytc = ytc_by_j[j]
                po = psum.tile([P, TQ], F32, name=f"po_{t}_{n2}", tag="acc",
                               bufs=3)
                for ck in range(NCK):
                    nc.tensor.matmul(
                        po, lhsT=ytc[ck][:, ts * P:(ts + 1) * P],
                        rhs=wpr_sb[ck][:, n2 * TQ:(n2 + 1) * TQ],
                        start=(ck == 0), stop=(ck == NCK - 1))
                ot = ostgp.tile([P, TQ], F32, name=f"ot_{t}_{n2}", tag="ot")
                if j < 2:  # ACT has slack early; it is the j>=2 bottleneck
                    nc.scalar.activation(ot, po,
                                         mybir.ActivationFunctionType.Copy)
                else:
                    nc.vector.tensor_copy(ot, po)
                if j == NJ - 1:
                    hw = TQ // 2
                    for q in range(2):
                        nc.sync.dma_start(
                            out=out[t * P:(t + 1) * P,
                                    n2 * TQ + q * hw:n2 * TQ + (q + 1) * hw],
                            in_=ot[:, q * hw:(q + 1) * hw])
                else:
                    nc.sync.dma_start(
                        out=out[t * P:(t + 1) * P, n2 * TQ:(n2 + 1) * TQ],
                        in_=ot)

            # ---- prologue: chunk-0 projections ----
            for m in MORDER:
                part_a(0, m)
            for ts in range(4):
                part_b(0, ts)

            # ---- pipelined main loop ----
            for j in range(NJ):
                if j + 1 < NJ:
                    emit_xt_dmas(j + 1)
                ytc_by_j[j] = [ytcp.tile([P, TQ], F32R, name=f"ytc_{j}_{u}",
                                         tag="ytc") for u in range(4)]
                for h in range(NHG):
                    py = attn_head_main(j, h)
                    if j + 1 < NJ:
                        part_a(j + 1, MORDER[h])
                        if h % 2 == 1:
                            part_b(j + 1, h // 2)
                    if j > 0:
                        ph3_group(j - 1, h)
                    attn_head_finish(j, h, py)

            for g in range(NHG):
                ph3_group(NJ - 1, g)

    nc.finalize()
    return nc


def _prep_inputs(x, w_attn, b_attn, w_proj):
    """Build the 8 per-core input maps from full inputs."""
    x = np.asarray(x, dtype=np.float32)
    w_attn = np.asarray(w_attn, dtype=np.float32)
    b_attn = np.asarray(b_attn, dtype=np.float32)
    w_proj = np.asarray(w_proj, dtype=np.float32)

    xTb = [_round_fp32r(x[b].T) for b in range(B)]
    tri_m = _round_fp32r(
        np.triu(np.ones((P, P), dtype=np.float32)))  # allow tq >= tk

    per_g = []
    for g in range(G):
        qc = slice(g * CL, (g + 1) * CL)
        kc = slice(C + g * CL, C + (g + 1) * CL)
        vc = slice(2 * C + g * CL, 2 * C + (g + 1) * CL)
        w_qk_g = _round_fp32r(
            np.concatenate([w_attn[:, qc], w_attn[:, kc]], axis=1))
        w_v_g = _round_fp32r(w_attn[:, vc])
        b_qk_g = np.ascontiguousarray(
            np.concatenate([b_attn[qc], b_attn[kc]]).reshape(NM, P).T)
        w_pr_g = _round_fp32r(w_proj[g * CL:(g + 1) * CL, :])
        per_g.append((w_qk_g, w_v_g, b_qk_g, w_pr_g))

    in_maps = []
    for c in range(8):
        b, g = c // G, c % G
        w_qk_g, w_v_g, b_qk_g, w_pr_g = per_g[g]
        in_maps.append({
            "xT": xTb[b], "w_qk": w_qk_g, "w_v": w_v_g, "b_qk": b_qk_g,
            "w_pr": w_pr_g, "tri": tri_m,
        })
    return in_maps


def _run(x, w_attn, b_attn, w_proj, b_proj, trace=False):
    from concourse.bass_utils import run_bass_kernel_spmd

    if "nc" not in _STATE:
        _STATE["nc"] = _build()
    nc = _STATE["nc"]

    in_maps = _prep_inputs(x, w_attn, b_attn, w_proj)
    res = run_bass_kernel_spmd(nc, in_maps, list(range(8)), trace=trace)

    b_proj = np.asarray(b_proj, dtype=np.float32)
    b_attn = np.asarray(b_attn, dtype=np.float32)
    w_proj_f = np.asarray(w_proj, dtype=np.float32)
    # v-projection bias is linear through w_proj: fold into the output bias
    bias_full = b_proj + b_attn[2 * C:] @ w_proj_f
    outp = np.empty((B, T, C), dtype=np.float32)
    for b in range(B):
        outp[b] = res.results[2 * b]["out"] + res.results[2 * b + 1]["out"]
    outp += bias_full
    return outp, res


def kernel(x, w_attn, b_attn, w_proj, b_proj):
    outp, _ = _run(x, w_attn, b_attn, w_proj, b_proj, trace=False)
    return outp
